# revision 18
# baseline (speedup 1.0000x reference)
"""Causal self-attention (S=2048, D=2048, H=32 heads, Dh=64) on 8 TRN2 cores.

Strategy: tensor-parallel over heads (4 heads/core), zero device collectives
(partial out-projections summed on host).

Per core:
  - QK projection in "transposed world": Q^T/K^T [64dh, S] per head via
    wqk columns as lhsT against xT chunks.
  - V projection in natural [key, dh] layout (keys on partitions) with a
    ones column per head for softmax denominators.
  - logits^T tiles [128 keys, freeq] = K_h^T.T @ Q_h^T with the free dim
    TRIMMED to the causal query range (queries >= 128*i for key chunk i);
    exp on ScalarE (scale=1/8 folded), 0/1 triangle mask multiply only on
    the single diagonal 128x128 sub-tile.
  - attn@V reoriented for full PE partition use: out[128 queries, 65] =
    probsT_tile.T @ [V|1], accumulated over key chunks (65-cycle matmuls
    instead of 512-cycle ones).  Column 64 = softmax denominator.
  - normalize via DVE reciprocal + free-dim broadcast multiply (denominator
    varies along partitions -> native broadcast; no DRAM round-trip).
  - attn^T for the out-projection produced by DMA xbar transposes (SP
    queue), not the PE.
  - out projection: attnT chunks as lhsT, Wout rows as rhs -> partial
    [S, D] per core stored as bf16, summed on host (+bias).  Emitted as
    fine-grained units used as PE filler inside the ScalarE-bound late
    attention blocks.

Self-contained: only concourse/numpy/ml_dtypes imports.
"""
from contextlib import ExitStack

import numpy as np
import orjson
import ml_dtypes

import concourse.bass as bass
import concourse.tile as tile
from concourse import mybir
from concourse.bass_utils import run_bass_kernel_spmd
from concourse.vector_clock import ScopedClock, VectorClock

S = 2048
D = 2048
DH = 64
NH = 32
NCORES = 8
HPC = NH // NCORES          # heads per core = 4
KC = D // 128               # contraction chunks = 16
BF16 = ml_dtypes.bfloat16
F32 = mybir.dt.float32
BF = mybir.dt.bfloat16


class _PatchedTileContext(tile.TileContext):
    """Workaround: walrus in this container allows only ONE sync wait per
    CTRL instruction; stock TileContext puts the whole global clock on the
    final drain.  Split the waits across single-wait SP nops instead."""

    def _drain_and_barrier(self, tick_clock, wait_clock):
        gc = tick_clock.global_clock
        n = len(gc)
        for proc in range(n):
            tick = gc[proc]
            if tick <= 0:
                continue
            vec = [0] * n
            vec[proc] = tick
            inst = self.nc.sync.nop(nofuse=True, hint=f"drain_wait_p{proc}")
            wait_clock.add_sem_waits(inst.ins, ScopedClock({None: VectorClock(vec)}))
        self.nc.sync.drain()
        self.nc.all_engine_barrier()
        assert self.sems is not None
        popped = self.nc._tile_sem_poison_stack.pop()
        assert popped is self._sem_poison
        self.nc.clear_and_free_semaphores(list(self.sems.allocated().values()))
        self.nc.all_engine_barrier()


def _legalize_sync_waits(nc, max_waits: int = 1):
    """Split any instruction with >max_waits sem waits into preceding
    same-engine single-wait NoOps (this walrus rejects multi-wait
    instructions of every class)."""
    j = orjson.loads(mybir.module_to_json_bytes(nc.m))
    counter = 0
    changed = False
    for func in j["functions"]:
        for blk in func["blocks"]:
            new_insts = []
            for inst in blk["instructions"]:
                si = inst.get("sync_info")
                waits = si.get("on_wait") if si else None
                if waits and len(waits) > max_waits:
                    changed = True
                    for w in waits[:-max_waits]:
                        counter += 1
                        new_insts.append({
                            "debug": inst.get("debug", 0),
                            "engine": inst["engine"],
                            "ins": [],
                            "name": f"LW-{counter}",
                            "opcode": "NoOp",
                            "outs": [],
                            "sync_info": {"on_update": [], "on_wait": [w]},
                            "text_hint": "legalize_wait",
                        })
                    si["on_wait"] = waits[-max_waits:]
                new_insts.append(inst)
            blk["instructions"] = new_insts
    if changed:
        nc.m = mybir.module_from_json_bytes(orjson.dumps(j))
    return nc


def build_nc(rep: int = 1, legalize: bool = True, dbg: bool = False):
    nc = bass.Bass()
    xT = nc.declare_dram_parameter("xT", [D, S], BF, isOutput=False)
    wqk = nc.declare_dram_parameter("wqk", [D, 2 * HPC * DH], BF, isOutput=False)
    wv = nc.declare_dram_parameter("wv", [D, HPC * DH], BF, isOutput=False)
    wout = nc.declare_dram_parameter("wout", [HPC * DH, D], BF, isOutput=False)
    trimask = nc.declare_dram_parameter("trimask", [128, 128], BF, isOutput=False)
    out = nc.declare_dram_parameter("out", [S, D], BF, isOutput=True)
    if dbg:
        dbg_qt = nc.declare_dram_parameter("dbg_qt", [128, 2 * S], BF, isOutput=True)
        dbg_kt = nc.declare_dram_parameter("dbg_kt", [128, 2 * S], BF, isOutput=True)
        dbg_v = nc.declare_dram_parameter("dbg_v", [128, KC * HPC * 65], BF, isOutput=True)
        dbg_at = nc.declare_dram_parameter("dbg_at", [128, 2 * S], BF, isOutput=True)

    with _PatchedTileContext(nc, pool_alloc_mode="queue") as tc, \
            ExitStack() as ctx:
        const = ctx.enter_context(tc.tile_pool(name="const", bufs=1))
        rec_pool = ctx.enter_context(tc.tile_pool(name="rec", bufs=3))
        an_pool = ctx.enter_context(tc.tile_pool(name="an", bufs=4))
        osb_pool = ctx.enter_context(tc.tile_pool(name="osb", bufs=6))

        wout_sb = const.tile([128, 2, S], BF)
        mask_sb = const.tile([128, 128], BF)
        qt_sb = const.tile([128, 2, S], BF)
        kt_sb = const.tile([128, 2, S], BF)
        # V in natural [key, dh] layout: [128 keys, chunk, head*65] with a
        # ones column at 64 of each head's 65-slot for the denominators.
        v_sb = const.tile([128, KC, HPC * 65], BF)
        # attn^T features: [128 feat, c (feature-128-chunk), S queries]
        attnT_sb = const.tile([128, 2, S], BF)
        scratch_sb = const.tile([1, 8], F32)

        # preload the exp activation table before any real work
        nc.vector.memset(scratch_sb[:], 0.0)
        nc.scalar.activation(scratch_sb[:], scratch_sb[:],
                             mybir.ActivationFunctionType.Exp)

        # ones columns for the softmax denominators
        nc.vector.memset(
            v_sb[:].rearrange("p k (h x) -> p k h x", h=HPC)[:, :, :, 64:65], 1.0
        )

        def q_off(s, i):
            """Absolute query start of the trimmed logits tile for key
            chunk i in query block s (512 queries per block)."""
            return max(512 * s, 128 * i)

        for _rep in range(rep):
            with tc.tile_pool(name="proj", bufs=1) as proj:
                xT_sb = proj.tile([128, KC, S], BF)
                wqk_sb = proj.tile([128, KC, 512], BF)
                wv_sb = proj.tile([128, KC, 256], BF)

                # ---- input DMAs; first chunks split small so the PE can
                # start early, wv before xT's second half (V-proj needs it
                # sooner than QK pass 1 needs late xT columns) ----
                nc.sync.dma_start(wqk_sb[:, 0, 0:128], wqk[0:128, 0:128])
                nc.sync.dma_start(xT_sb[:, 0, 0:512], xT[0:128, 0:512])
                nc.sync.dma_start(wqk_sb[:, 0, 128:512], wqk[0:128, 128:512])
                nc.sync.dma_start(xT_sb[:, 0, 512:1024], xT[0:128, 512:1024])
                for k in range(1, KC):
                    nc.sync.dma_start(wqk_sb[:, k, :],
                                      wqk[k * 128:(k + 1) * 128, :])
                    nc.sync.dma_start(xT_sb[:, k, 0:1024],
                                      xT[k * 128:(k + 1) * 128, 0:1024])
                for k in range(KC):
                    nc.gpsimd.dma_start(wv_sb[:, k, :],
                                        wv[k * 128:(k + 1) * 128, :])
                for k in range(KC):
                    nc.sync.dma_start(xT_sb[:, k, 1024:2048],
                                      xT[k * 128:(k + 1) * 128, 1024:2048])
                nc.sync.dma_start(mask_sb[:], trimask[:])
                for c in range(2):
                    nc.sync.dma_start(wout_sb[:, c, :],
                                      wout[c * 128:(c + 1) * 128, :])

                def emit_qk_pass0(pool):
                    """Q^T/K^T for queries 0:1024, 8 psum accumulators for
                    ILP during the input-DMA ramp."""
                    pss = {}
                    for m in range(4):
                        for s in (0, 1):
                            pss[(m, s)] = pool.tile(
                                [128, 512], F32, name=f"qk_ps_{m}_{s}", tag="qk0")
                    for k in range(KC):
                        for m in range(4):
                            for s in (0, 1):
                                nc.tensor.matmul(
                                    pss[(m, s)],
                                    wqk_sb[:, k, m * 128:(m + 1) * 128],
                                    xT_sb[:, k, s * 512:(s + 1) * 512],
                                    start=(k == 0), stop=(k == KC - 1),
                                    skip_group_check=True,
                                )
                    for m in range(4):
                        for s in (0, 1):
                            dest = qt_sb if m < 2 else kt_sb
                            nc.vector.tensor_copy(
                                dest[:, m % 2, s * 512:(s + 1) * 512],
                                pss[(m, s)])

                def emit_qk1_chain(pool, m, s):
                    """One (m, s) accumulation chain of QK pass 1 (PE
                    filler unit: 16 matmuls, psum bufs rotate)."""
                    ps = pool.tile([128, 512], F32, name=f"qk1_{m}_{s}",
                                   tag="qk1")
                    for k in range(KC):
                        nc.tensor.matmul(
                            ps,
                            wqk_sb[:, k, m * 128:(m + 1) * 128],
                            xT_sb[:, k, s * 512:(s + 1) * 512],
                            start=(k == 0), stop=(k == KC - 1),
                            skip_group_check=True,
                        )
                    dest = qt_sb if m < 2 else kt_sb
                    nc.vector.tensor_copy(
                        dest[:, m % 2, s * 512:(s + 1) * 512], ps)

                def emit_v_chain(pool, sv):
                    """V projection for key chunk sv -> v_sb natural
                    layout."""
                    psv = pool.tile([128, 256], F32, name=f"v_{sv}", tag="v")
                    for k in range(KC):
                        nc.tensor.matmul(
                            psv,
                            xT_sb[:, k, sv * 128:(sv + 1) * 128],
                            wv_sb[:, k, :],
                            start=(k == 0), stop=(k == KC - 1),
                            skip_group_check=True,
                        )
                    nc.vector.tensor_copy(
                        v_sb[:, sv, :].rearrange(
                            "p (h x) -> p h x", h=HPC)[:, :, 0:64],
                        psv[:].rearrange("p (h x) -> p h x", h=HPC),
                    )

                def emit_logits(lg_pool, pp, s, i):
                    """Trimmed logits + exp (+ diagonal mask) for key chunk
                    i of query block s.  Returns {hc: probs_tile}."""
                    qoff = q_off(s, i)
                    freeq = 512 * (s + 1) - qoff
                    res = {}
                    for hc in range(2):
                        lg = lg_pool.tile([128, 1024], F32, name="lg",
                                          tag="lg")
                        for u in range(2):
                            hp = u * 64
                            nc.tensor.matmul(
                                lg[:, u * 512:u * 512 + freeq],
                                kt_sb[hp:hp + 64, hc, i * 128:(i + 1) * 128],
                                qt_sb[hp:hp + 64, hc, qoff:qoff + freeq],
                                start=True, stop=True,
                                skip_group_check=True,
                            )
                        probs = pp.tile(
                            [128, 2, 512], BF, name="probs", tag="probs")
                        nc.scalar.activation(
                            probs[:, :, 0:freeq],
                            lg[:].rearrange("p (u q) -> p u q", u=2)[:, :, 0:freeq],
                            mybir.ActivationFunctionType.Exp, scale=0.125,
                        )
                        if i >= 4 * s:  # diagonal tile: triangle mask
                            nc.vector.tensor_mul(
                                probs[:, :, 0:128],
                                probs[:, :, 0:128],
                                mask_sb[:, None, :].to_broadcast((128, 2, 128)),
                            )
                        res[hc] = probs
                    return res

                def emit_av_step(at_ps, probs_i, s, i, jq):
                    """attn@V chain step i for query chunk jq (4 heads).
                    Only the FIRST matmul of the chain carries start=True:
                    start marks the whole 2KB psum zero-region pending-zero,
                    so sibling head-groups in the same bank must NOT restart
                    it -- they write into the pending region (hw lazy
                    zeroing) and accumulate from step 1 on."""
                    qoff = q_off(s, i)
                    qrel = jq * 128 - qoff
                    for h in range(HPC):
                        hc, u = h // 2, h % 2
                        nc.tensor.matmul(
                            at_ps[:, h * 65:(h + 1) * 65],
                            probs_i[hc][:, u, qrel:qrel + 128],
                            v_sb[:, i, h * 65:(h + 1) * 65],
                            start=(i == 0 and h == 0), stop=(i == jq),
                            skip_group_check=True,
                        )

                def emit_finish_q(at_ps, jq):
                    """Normalize chunk jq's attn rows and xbar-transpose
                    them into attnT_sb."""
                    atv = at_ps[:].rearrange("p (h x) -> p h x", x=65)
                    rec = rec_pool.tile([128, 4], F32, tag="rec")
                    nc.vector.reciprocal(rec[:], atv[:, :, 64])
                    an = an_pool.tile([128, 256], BF, tag="an", name="an")
                    nc.vector.tensor_mul(
                        an[:].rearrange("p (h x) -> p h x", h=4),
                        atv[:, :, 0:64],
                        rec[:, :, None].to_broadcast((128, 4, 64)),
                    )
                    for c in range(2):
                        nc.sync.dma_start_transpose(
                            attnT_sb[:, c, jq * 128:(jq + 1) * 128],
                            an[:, c * 128:(c + 1) * 128],
                        )

                def emit_block_paired(lg_pool, at_pool, pp, s, filler):
                    """Attention query block s, attn@V chains two at a time
                    (2 psum banks).  `filler` is a list of zero-arg PE-work
                    emitters popped throughout to keep the PE fed while the
                    ScalarE works through the exps."""
                    def fill(n=1):
                        for _ in range(n):
                            if filler:
                                filler.pop(0)()
                    n_i = 4 * s + 4
                    probs_i = []
                    for i in range(n_i):
                        probs_i.append(emit_logits(lg_pool, pp, s, i))
                        fill()
                    for jq in range(4 * s, 4 * s + 4):
                        at1 = at_pool.tile([128, HPC * 65], F32,
                                           name=f"at{jq}", tag="at")
                        for i in range(jq + 1):
                            emit_av_step(at1, probs_i[i], s, i, jq)
                            fill()
                        emit_finish_q(at1, jq)

                # ---- schedule ----
                with tc.tile_pool(name="qkps", bufs=8, space="PSUM") as qkps:
                    emit_qk_pass0(qkps)

                with (
                    tc.tile_pool(name="lg", bufs=2, space="PSUM") as lg,
                    tc.tile_pool(name="atp", bufs=2, space="PSUM") as atp,
                    tc.tile_pool(name="probs1", bufs=18) as pp1,
                ):
                    with tc.tile_pool(name="vps", bufs=2, space="PSUM") as vps:
                        # V for key chunks 0..3 unlocks attention block 0
                        for sv in range(4):
                            emit_v_chain(vps, sv)
                        vfill = [
                            (lambda sv=sv: emit_v_chain(vps, sv))
                            for sv in range(4, KC)
                        ]
                        emit_block_paired(lg, atp, pp1, 0, vfill[:3])
                        for f in vfill[3:]:
                            f()
                    with tc.tile_pool(name="qk1", bufs=2, space="PSUM") as qk1:
                        qfill = [
                            (lambda m=m, s=s: emit_qk1_chain(qk1, m, s))
                            for s in (2, 3) for m in range(4)
                        ]
                        emit_block_paired(lg, atp, pp1, 1, qfill[:2])
                        for f in qfill[2:]:
                            f()

            # ---- blocks 2,3: one software-pipelined region.  Logits are
            # emitted ahead (lead-in) so the ScalarE exp stream never
            # starves; out-projection units act as PE filler but only
            # become eligible once their attn^T chunk's transpose has been
            # emitted (deps are computed at emission time).  Combined
            # ScalarE work (~44us) < PE work (~50us) -> PE-bound. ----
            with (
                tc.tile_pool(name="lg2", bufs=2, space="PSUM") as lg2,
                tc.tile_pool(name="atp2", bufs=2, space="PSUM") as atp2,
                tc.tile_pool(name="ops", bufs=2, space="PSUM") as ops,
                tc.tile_pool(name="probs2", bufs=50) as pp2,
            ):
                def make_units(mo):
                    units = []
                    for n in range(4):
                        def u(mo=mo, n=n):
                            pso = ops.tile([128, 512], F32, name="pso",
                                           tag="pso")
                            for c in range(2):
                                nc.tensor.matmul(
                                    pso[:],
                                    attnT_sb[:, c, mo * 128:(mo + 1) * 128],
                                    wout_sb[:, c, n * 512:(n + 1) * 512],
                                    start=(c == 0), stop=(c == 1),
                                    skip_group_check=True,
                                )
                            ob = osb_pool.tile([128, 512], BF, tag="ob",
                                               name="ob")
                            nc.vector.tensor_copy(ob[:], pso[:])
                            nc.sync.dma_start(
                                out[mo * 128:(mo + 1) * 128,
                                    n * 512:(n + 1) * 512], ob[:])
                        units.append(u)
                    return units

                oq = []
                for mo in range(8):
                    oq.extend(make_units(mo))

                probs2 = {}
                Lseq = [(2, i) for i in range(12)] + [(3, i) for i in range(16)]
                li = 0

                # time accounting: pop op units whenever emitted PE work
                # falls behind the ScalarE exp stream, so the PE never
                # starves while saving units for the Act-bound tail.
                ns = {"act": 0.0, "pe": 0.0}

                def popO():
                    while oq and ns["pe"] < ns["act"] - 1000.0:
                        ns["pe"] += 440.0
                        oq.pop(0)()

                def advance_LA(n=1):
                    nonlocal li
                    for _ in range(n):
                        if li < len(Lseq):
                            s_, i_ = Lseq[li]
                            freeq = 512 * (s_ + 1) - q_off(s_, i_)
                            ns["act"] += 2 * (2 * freeq * 0.853 + 120.0)
                            ns["pe"] += 4 * freeq * 0.4167
                            probs2[(s_, i_)] = emit_logits(lg2, pp2, s_, i_)
                            li += 1

                advance_LA(6)
                for s, jq in [(2, j) for j in range(8, 12)] + \
                             [(3, j) for j in range(12, 16)]:
                    at1 = atp2.tile([128, HPC * 65], F32,
                                    name=f"at{jq}", tag="at")
                    for i in range(jq + 1):
                        advance_LA(1)
                        assert (s, i) in probs2
                        emit_av_step(at1, probs2[(s, i)], s, i, jq)
                        ns["pe"] += 110.0
                        popO()
                    emit_finish_q(at1, jq)
                    oq.extend(make_units(jq))
                while oq:
                    oq.pop(0)()
                if dbg:
                    nc.sync.dma_start(
                        dbg_qt[:], qt_sb[:].rearrange("p a b -> p (a b)"))
                    nc.sync.dma_start(
                        dbg_kt[:], kt_sb[:].rearrange("p a b -> p (a b)"))
                    nc.sync.dma_start(
                        dbg_v[:], v_sb[:].rearrange("p a b -> p (a b)"))
                    nc.sync.dma_start(
                        dbg_at[:], attnT_sb[:].rearrange("p a b -> p (a b)"))

    if legalize:
        _legalize_sync_waits(nc)
    return nc


_NC_CACHE = None


def _get_nc():
    global _NC_CACHE
    if _NC_CACHE is None:
        _NC_CACHE = build_nc()
    return _NC_CACHE


def make_inputs(x, Wqkv, Wout, bias):
    xT = np.ascontiguousarray(x.T).astype(BF16)
    r = np.arange(128)[:, None]
    c = np.arange(128)[None, :]
    tri = (c >= r).astype(np.float32).astype(BF16)
    in_maps = []
    for core in range(NCORES):
        h0 = core * HPC * DH          # 256 cols per core per q/k/v section
        wq = Wqkv[:, h0:h0 + HPC * DH]
        wk = Wqkv[:, D + h0:D + h0 + HPC * DH]
        wv_ = Wqkv[:, 2 * D + h0:2 * D + h0 + HPC * DH]
        in_maps.append({
            "xT": xT,
            "wqk": np.concatenate([wq, wk], axis=1).astype(BF16),
            "wv": wv_.astype(BF16),
            "wout": Wout[h0:h0 + HPC * DH, :].astype(BF16),
            "trimask": tri,
        })
    return in_maps


def kernel(x, Wqkv, Wout, bias, _trace=False, _trace_kwargs=None):
    x = np.asarray(x)
    Wqkv = np.asarray(Wqkv)
    Wout = np.asarray(Wout)
    bias = np.asarray(bias)
    nc = _get_nc()
    in_maps = make_inputs(x, Wqkv, Wout, bias)
    res = run_bass_kernel_spmd(
        nc, in_maps, core_ids=list(range(NCORES)),
        trace=_trace, **(_trace_kwargs or {}),
    )
    acc = np.zeros((S, D), np.float64)
    for core in range(NCORES):
        acc += res.results[core]["out"].astype(np.float64)
    out = (acc + bias.astype(np.float64)[None, :]).astype(np.float32)
    if _trace:
        kernel._last_result = res
    return out
